# revision 10
# baseline (speedup 1.0000x reference)
"""Trainium2 Bass kernel for nn_Decoder (sparse top-8 attention decoder layer).

Contract: kernel(**inputs) takes the FULL unsharded inputs (B=2, S=2048,
D=1024, H=16 heads, top-8 sparse causal attention + ReZero FFN) and returns
the full [2, 2048, 1024] fp32 output.

Sharding: sequence-parallel over 8 cores, no collectives. Each core owns 4
query tiles of 128 rows from one batch, one tile from each causal-length
quartile so the SPMD program is uniform and balanced: core i (b = i//4,
m = i%4) owns absolute q-tiles {m, m+4, m+8, m+12}; q-tile m+4s runs in
"slot" s scanning a key window of 512*(s+1) keys (true causal window is
smaller; the remainder is masked additively). Every core redundantly
computes K/V projections for all 2048 keys of its batch.

Numerics: Q/K projections and Q.K^T use a 3-term bf16 hi/lo split
(error ~2^-17) because top-8 selection is sensitive to score noise near the
8th/9th-largest boundary (plain bf16 flips ~5% of rows and fails).
V/Wo/FFN run in bf16 with fp32 accumulation. Softmax runs on the 8
surviving scores only (exp of masked scores underflows to exactly 0,
matching the reference's -10000 masking); attn@V is a dma_gather of the 8
selected V rows per query plus a small weighted reduction instead of a
dense [S,S] @ [S,DK] matmul.
"""

import math
from contextlib import ExitStack

import ml_dtypes
import numpy as np

B, S, D, H, TOPK, DK = 2, 2048, 1024, 16, 8, 64
NC = 8          # cores
SQ = 512        # query rows per core (4 tiles of 128)
KE = 2048       # key window per core (uniform)
NT = 4          # q-tile slots per core; slot s scans 512*(s+1) keys
NEGBIG = -100000.0

_compiled = None


def _bf16_split(a):
    hi = a.astype(ml_dtypes.bfloat16)
    lo = (a - hi.astype(np.float32)).astype(ml_dtypes.bfloat16)
    return hi, lo


def _build_program():
    import concourse.tile as tile
    from concourse import bacc, mybir

    f32 = mybir.dt.float32
    bf16 = mybir.dt.bfloat16
    u16 = mybir.dt.uint16

    nc = bacc.Bacc("TRN2", target_bir_lowering=False, debug=False, num_devices=NC,
                   num_swdge_queues=4)

    def din(name, shape, dt):
        return nc.dram_tensor(name, shape, dt, kind="ExternalInput").ap()

    aps = {
        "xk_hi": din("xk_hi", [D, KE], bf16),
        "xk_lo": din("xk_lo", [D, KE], bf16),
        "xq_hi": din("xq_hi", [D, SQ], bf16),
        "xq_lo": din("xq_lo", [D, SQ], bf16),
        "wq_hi": din("wq_hi", [8, 128, D], bf16),
        "wq_lo": din("wq_lo", [8, 128, D], bf16),
        "wk_hi": din("wk_hi", [8, 128, D], bf16),
        "wk_lo": din("wk_lo", [8, 128, D], bf16),
        "icidx": din("icidx", [128, 4], u16),
        "c16": din("c16", [128, 128], f32),
        "bq_row": din("bq_row", [1, D], bf16),
        "bk_row": din("bk_row", [1, D], bf16),
        "wv": din("wv", [D, D], bf16),
        "bv_row": din("bv_row", [1, D], bf16),
        "wo": din("wo", [D, D], bf16),
        "wo_b": din("wo_b", [1, D], bf16),
        "w1": din("w1", [D, 4 * D], bf16),
        "b1t": din("b1t", [128, 32], f32),
        "w2": din("w2", [4 * D, D], bf16),
        "b2_row": din("b2_row", [1, D], bf16),
        "maskd": din("maskd", [NT, 128, 512], f32),
        "ident": din("ident", [128, 128], bf16),
        "y": nc.dram_tensor("y", [SQ, D], f32, kind="ExternalOutput").ap(),
        "v_dram": nc.dram_tensor("v_dram", [KE, D], f32).ap(),
    }

    with tile.TileContext(nc) as tc:
        _emit(nc, tc, mybir, aps)
    nc.compile()
    return nc


def _emit(nc, tc, mybir, t):
    f32 = mybir.dt.float32
    bf16 = mybir.dt.bfloat16
    u16 = mybir.dt.uint16
    i16 = mybir.dt.int16
    AF = mybir.ActivationFunctionType
    OP = mybir.AluOpType

    y_d, v_dram = t["y"], t["v_dram"]

    with ExitStack() as ctx:
        # ---------------- constants (live whole kernel) -----------------
        const = ctx.enter_context(tc.tile_pool(name="const", bufs=1))
        ones = const.tile([1, 512], bf16)
        nc.vector.memset(ones[:], 1.0)
        identt = const.tile([128, 128], bf16)
        nc.sync.dma_start(identt[:], t["ident"][:, :])
        brow = {}
        for nm, key in [("bq", "bq_row"), ("bk", "bk_row"), ("bv", "bv_row"),
                        ("wo_b", "wo_b"), ("b2", "b2_row")]:
            r = const.tile([1, D], bf16, tag=f"bias_{nm}")
            nc.sync.dma_start(r[:], t[key][:, :])
            brow[nm] = r
        b1t = const.tile([128, 32], f32)
        nc.sync.dma_start(b1t[:], t["b1t"][:, :])
        icidx = const.tile([128, 4], u16)
        nc.sync.dma_start(icidx[:], t["icidx"][:, :])
        c16 = const.tile([128, 128], f32)
        nc.sync.dma_start(c16[:], t["c16"][:, :])
        maskd = const.tile([128, NT * 512], f32)
        nc.sync.dma_start(maskd[:].rearrange("p (s c) -> p s c", s=NT),
                          t["maskd"].rearrange("s p c -> p s c"))

        # ctx outputs of attention, consumed by Wo phase
        ctxp = ctx.enter_context(tc.tile_pool(name="ctxT", bufs=1))
        ctxT = [ctxp.tile([128, SQ], bf16, tag=f"ctxT{hp}", name=f"ctxT{hp}")
                for hp in range(8)]

        with tc.tile_pool(name="x", bufs=1) as xpool:
            xkh, xkl, xqh, xql = [], [], [], []
            for c in range(8):
                sl = slice(c * 128, (c + 1) * 128)
                th = xpool.tile([128, KE], bf16, tag=f"xkh{c}")
                nc.sync.dma_start(th[:], t["xk_hi"][sl, :])
                xkh.append(th)
                tl = xpool.tile([128, KE], bf16, tag=f"xkl{c}")
                nc.sync.dma_start(tl[:], t["xk_lo"][sl, :])
                xkl.append(tl)
                qh = xpool.tile([128, SQ], bf16, tag=f"xqh{c}")
                nc.sync.dma_start(qh[:], t["xq_hi"][sl, :])
                xqh.append(qh)
                ql = xpool.tile([128, SQ], bf16, tag=f"xql{c}")
                nc.sync.dma_start(ql[:], t["xq_lo"][sl, :])
                xql.append(ql)

            # ------------ phase V: v = x @ Wv.T + bv -> v_dram ----------
            with tc.tile_pool(name="wvp", bufs=1) as wvp, \
                 tc.tile_pool(name="vps", bufs=4, space="PSUM") as vps, \
                 tc.tile_pool(name="vsb", bufs=4) as vsbp:
                wvt = []
                for c in range(8):
                    w = wvp.tile([128, D], bf16, tag=f"wv{c}")
                    nc.sync.dma_start(w[:], t["wv"][c * 128:(c + 1) * 128, :])
                    wvt.append(w)
                for rt in range(16):
                    for oc in range(2):
                        ps = vps.tile([128, 512], f32)
                        for c in range(8):
                            nc.tensor.matmul(
                                ps[:], xkh[c][:, rt * 128:(rt + 1) * 128],
                                wvt[c][:, oc * 512:(oc + 1) * 512],
                                start=(c == 0), stop=False)
                        nc.tensor.matmul(
                            ps[:], ones[0:1, 0:128],
                            brow["bv"][0:1, oc * 512:(oc + 1) * 512],
                            start=False, stop=True)
                        vs = vsbp.tile([128, 512], f32)
                        nc.scalar.copy(vs[:], ps[:])
                        nc.sync.dma_start(
                            v_dram[rt * 128:(rt + 1) * 128,
                                   oc * 512:(oc + 1) * 512], vs[:])

            # ------------ attention loop over head-pairs ----------------
            with tc.tile_pool(name="att", bufs=2) as att, \
                 tc.tile_pool(name="attps", bufs=1, space="PSUM") as attps, \
                 tc.tile_pool(name="scps", bufs=2, space="PSUM") as scps, \
                 tc.tile_pool(name="scores", bufs=2) as scores_p, \
                 tc.tile_pool(name="small", bufs=4) as small, \
                 tc.tile_pool(name="gather", bufs=3) as gat:
                for hp in range(8):
                    cs, ce = hp * 128, (hp + 1) * 128
                    wkh = att.tile([128, 1024], bf16, tag="wkh")
                    wkl = att.tile([128, 1024], bf16, tag="wkl")
                    wqh = att.tile([128, 1024], bf16, tag="wqh")
                    wql = att.tile([128, 1024], bf16, tag="wql")
                    nc.sync.dma_start(wkh[:], t["wk_hi"][hp, :, :])
                    nc.sync.dma_start(wkl[:], t["wk_lo"][hp, :, :])
                    nc.sync.dma_start(wqh[:], t["wq_hi"][hp, :, :])
                    nc.sync.dma_start(wql[:], t["wq_lo"][hp, :, :])

                    # K projection (3-pass hi/lo split + bias); kc-inner so the
                    # stationary weight slice is reused across 4 key chunks
                    khi = att.tile([128, KE], bf16, tag="khi")
                    klo = att.tile([128, KE], bf16, tag="klo")
                    kps = [attps.tile([128, 512], f32, tag=f"b{kc}",
                                      name=f"kps{kc}")
                           for kc in range(4)]
                    for pi, (wt, xt) in enumerate(
                            ((wkh, xkh), (wkl, xkh), (wkh, xkl))):
                        for c in range(8):
                            for kc in range(4):
                                nc.tensor.matmul(
                                    kps[kc][:], wt[:, c * 128:(c + 1) * 128],
                                    xt[c][:, kc * 512:(kc + 1) * 512],
                                    start=(pi == 0 and c == 0), stop=False)
                    for kc in range(4):
                        ksl = slice(kc * 512, (kc + 1) * 512)
                        nc.tensor.matmul(kps[kc][:], brow["bk"][0:1, cs:ce],
                                         ones[0:1, 0:512], start=False, stop=True)
                        nc.scalar.copy(khi[:, ksl], kps[kc][:])
                        nc.vector.tensor_sub(klo[:, ksl], kps[kc][:],
                                             khi[:, ksl])

                    # Q projection
                    qhi = att.tile([128, SQ], bf16, tag="qhi")
                    qlo = att.tile([128, SQ], bf16, tag="qlo")
                    ps = attps.tile([128, 512], f32, tag="b0")
                    first = True
                    for wt, xt in ((wqh, xqh), (wql, xqh), (wqh, xql)):
                        for c in range(8):
                            nc.tensor.matmul(
                                ps[:], wt[:, c * 128:(c + 1) * 128], xt[c][:, :],
                                start=first, stop=False)
                            first = False
                    nc.tensor.matmul(ps[:], brow["bq"][0:1, cs:ce],
                                     ones[0:1, 0:512], start=False, stop=True)
                    nc.scalar.copy(qhi[:, :], ps[:])
                    nc.vector.tensor_sub(qlo[:, :], ps[:], qhi[:, :])

                    for s in range(NT):
                        L = 512 * (s + 1)
                        qsl = slice(128 * s, 128 * (s + 1))
                        scs = [scores_p.tile([128, KE], f32, tag=f"scores{h2}",
                                             name=f"sc{h2}")
                               for h2 in range(2)]
                        for kc in range(s + 1):
                            ksl = slice(kc * 512, (kc + 1) * 512)
                            sps = [scps.tile([128, 512], f32, tag=f"scoreps{h2}",
                                             name=f"sp{h2}")
                                   for h2 in range(2)]
                            # interleaved: the two heads hit disjoint PE row
                            # groups (partitions 0-63 / 64-127) and overlap
                            for qa, ka in ((qhi, khi), (qhi, klo), (qlo, khi)):
                                for h2 in range(2):
                                    hsl = slice(64 * h2, 64 * h2 + 64)
                                    nc.tensor.matmul(
                                        sps[h2][:], qa[hsl, qsl], ka[hsl, ksl],
                                        start=(qa is qhi and ka is khi),
                                        stop=(qa is qlo))
                            for h2 in range(2):
                                if kc == s:
                                    nc.vector.tensor_add(
                                        scs[h2][:, ksl], sps[h2][:],
                                        maskd[:, s * 512:(s + 1) * 512])
                                else:
                                    nc.scalar.copy(scs[h2][:, ksl], sps[h2][:])
                        for h2 in range(2):
                            h = 2 * hp + h2
                            hsl = slice(64 * h2, 64 * h2 + 64)
                            sc = scs[h2]
                            m8 = small.tile([128, 8], f32, tag="m8")
                            nc.vector.max(m8[:], sc[:, 0:L])
                            idx = small.tile([128, 8], u16, tag="idx")
                            nc.vector.max_index(idx[:], m8[:], sc[:, 0:L])
                            w8 = small.tile([128, 8], f32, tag="w8")
                            z = small.tile([128, 1], f32, tag="z")
                            nc.scalar.activation(w8[:], m8[:], AF.Exp,
                                                 scale=1.0 / math.sqrt(DK),
                                                 accum_out=z[:])
                            rz = small.tile([128, 1], f32, tag="rz")
                            nc.vector.reciprocal(rz[:], z[:])
                            wn = small.tile([128, 8], f32, tag="wn")
                            nc.vector.tensor_mul(wn[:], w8[:],
                                                 rz[:].broadcast_to([128, 8]))
                            # build the wrapped+replicated index layout fully
                            # on-chip: idx -> f32, per-16-partition-group shift
                            # via indirect_copy (host-const indices), then an
                            # exact 0/1-selector fp32 matmul that also
                            # replicates across the 8 partition groups
                            mf = small.tile([128, 16], f32, tag="mf")
                            nc.vector.tensor_copy(mf[:, 0:8], idx[:])
                            nc.vector.memset(mf[:, 8:9], 0.0)
                            mbig = small.tile([128, 64], f32, tag="mbig")
                            nc.gpsimd.indirect_copy(
                                mbig[:], mf[:], icidx[:],
                                i_know_ap_gather_is_preferred=True)
                            wps = attps.tile([128, 64], f32, tag="b2",
                                             name="wps")
                            nc.tensor.matmul(wps[:], c16[:], mbig[:],
                                             start=True, stop=True)
                            wrap = small.tile([128, 64], i16, tag="wrap")
                            nc.vector.tensor_copy(wrap[:], wps[:])
                            gth = gat.tile([128, 8, 64], f32, tag="gth")
                            nc.gpsimd.dma_gather(
                                out_ap=gth[:],
                                in_ap=v_dram[:, 64 * h:64 * (h + 1)],
                                idxs_ap=wrap[:],
                                num_idxs=1024, num_idxs_reg=1024,
                                elem_size=64, elem_step=D,
                                queue_num=(h * NT + s) % 4,
                            )
                            prod = gat.tile([128, 8, 64], f32, tag="prod")
                            nc.vector.tensor_mul(
                                prod[:], gth[:],
                                wn[:].unsqueeze(2).broadcast_to([128, 8, 64]))
                            cx = small.tile([128, 64], f32, tag="cx")
                            nc.vector.tensor_reduce(
                                cx[:], prod[:].rearrange("p j d -> p d j"),
                                axis=mybir.AxisListType.X, op=OP.add)
                            cxb = small.tile([128, 64], bf16, tag="cxb")
                            nc.scalar.copy(cxb[:], cx[:])
                            ct = attps.tile([64, 128], bf16, tag="b3",
                                            name="ct")
                            nc.tensor.transpose(ct[:], cxb[:], identt[:])
                            nc.scalar.copy(ctxT[hp][hsl, qsl], ct[:])

        # ---------------- Wo: hT = (2 g1)(ctx Wo.T + bo), transposed ----
        hTp = ctx.enter_context(tc.tile_pool(name="hT", bufs=1))
        hT = []
        with tc.tile_pool(name="wop", bufs=1) as wop, \
             tc.tile_pool(name="wops", bufs=4, space="PSUM") as wops:
            wot = []
            for c in range(8):
                w = wop.tile([128, D], bf16, tag=f"wo{c}")
                nc.sync.dma_start(w[:], t["wo"][c * 128:(c + 1) * 128, :])
                wot.append(w)
            for ot in range(8):
                ps = wops.tile([128, 512], f32)
                for c in range(8):
                    nc.tensor.matmul(ps[:], wot[c][:, ot * 128:(ot + 1) * 128],
                                     ctxT[c][:, :], start=(c == 0), stop=False)
                nc.tensor.matmul(
                    ps[:], brow["wo_b"][0:1, ot * 128:(ot + 1) * 128],
                    ones[0:1, 0:512], start=False, stop=True)
                ht = hTp.tile([128, SQ], bf16, tag=f"hT{ot}")
                nc.scalar.copy(ht[:], ps[:])
                hT.append(ht)

        # ---------------- FFN mm1 + gelu --------------------------------
        gTp = ctx.enter_context(tc.tile_pool(name="gT", bufs=1))
        gT = []
        with tc.tile_pool(name="w1p", bufs=1) as w1p, \
             tc.tile_pool(name="f1ps", bufs=4, space="PSUM") as f1ps:
            w1t = []
            for c in range(8):
                w = w1p.tile([128, 4 * D], bf16, tag=f"w1{c}")
                nc.sync.dma_start(w[:], t["w1"][c * 128:(c + 1) * 128, :])
                w1t.append(w)
            for ft in range(32):
                ps = f1ps.tile([128, 512], f32)
                for c in range(8):
                    nc.tensor.matmul(ps[:], w1t[c][:, ft * 128:(ft + 1) * 128],
                                     hT[c][:, :], start=(c == 0), stop=(c == 7))
                g = gTp.tile([128, SQ], bf16, tag=f"gT{ft}")
                nc.scalar.activation(g[:], ps[:], AF.Gelu,
                                     bias=b1t[:, ft:ft + 1], scale=1.0)
                gT.append(g)

        # ---------------- FFN mm2 + bias + out --------------------------
        with tc.tile_pool(name="w2p", bufs=1) as w2p, \
             tc.tile_pool(name="yps", bufs=4, space="PSUM") as yps, \
             tc.tile_pool(name="ysb", bufs=4) as ysbp:
            w2t = []
            for fc in range(32):
                w = w2p.tile([128, D], bf16, tag=f"w2{fc}")
                nc.sync.dma_start(w[:], t["w2"][fc * 128:(fc + 1) * 128, :])
                w2t.append(w)
            for qt in range(4):
                for oc in range(2):
                    ps = yps.tile([128, 512], f32)
                    for fc in range(32):
                        nc.tensor.matmul(
                            ps[:], gT[fc][:, qt * 128:(qt + 1) * 128],
                            w2t[fc][:, oc * 512:(oc + 1) * 512],
                            start=(fc == 0), stop=False)
                    nc.tensor.matmul(
                        ps[:], ones[0:1, 0:128],
                        brow["b2"][0:1, oc * 512:(oc + 1) * 512],
                        start=False, stop=True)
                    ys = ysbp.tile([128, 512], f32)
                    nc.scalar.copy(ys[:], ps[:])
                    nc.sync.dma_start(
                        y_d[qt * 128:(qt + 1) * 128, oc * 512:(oc + 1) * 512],
                        ys[:])


def _prep_inputs(x, Wq, bq, Wk, bk, Wv, bv, Wo, bo, g1, W1, b1, W2, b2, g2):
    f32 = np.float32
    bf = ml_dtypes.bfloat16
    x = np.asarray(x, f32)
    g1 = float(np.asarray(g1))
    g2 = float(np.asarray(g2))

    def _hp_major(w):
        # [c*128+p, hp*128+q] -> [hp, p, c*128+q]
        return np.ascontiguousarray(
            w.reshape(8, 128, 8, 128).transpose(2, 1, 0, 3).reshape(8, 128, D))

    wq_hi, wq_lo = _bf16_split(np.ascontiguousarray(np.asarray(Wq, f32).T))
    wk_hi, wk_lo = _bf16_split(np.ascontiguousarray(np.asarray(Wk, f32).T))
    # indirect_copy indices: per 16-partition group g, list[8j+g'] = j if
    # g'==g else 8 (points at the zero pad col). Wrapped [16, 4] per group.
    ic = np.zeros((8, 64), np.uint16)
    for g in range(8):
        for j in range(8):
            for gp in range(8):
                ic[g, 8 * j + gp] = j if gp == g else 8
    icidx = np.zeros((128, 4), np.uint16)
    for g in range(8):
        for i in range(64):
            icidx[16 * g + i % 16, i // 16] = ic[g, i]
    p = np.arange(128)
    c16 = (p[:, None] % 16 == p[None, :] % 16).astype(f32)
    shared = {
        "wq_hi": _hp_major(wq_hi), "wq_lo": _hp_major(wq_lo),
        "wk_hi": _hp_major(wk_hi), "wk_lo": _hp_major(wk_lo),
        "icidx": icidx, "c16": c16,
        "bq_row": np.asarray(bq, f32).reshape(1, D).astype(bf),
        "bk_row": np.asarray(bk, f32).reshape(1, D).astype(bf),
        "wv": np.ascontiguousarray(np.asarray(Wv, f32).T).astype(bf),
        "bv_row": np.asarray(bv, f32).reshape(1, D).astype(bf),
        "wo": np.ascontiguousarray(2 * g1 * np.asarray(Wo, f32).T).astype(bf),
        "wo_b": (2 * g1 * np.asarray(bo, f32)).reshape(1, D).astype(bf),
        "w1": np.ascontiguousarray(np.asarray(W1, f32).T).astype(bf),
        "b1t": np.asarray(b1, f32).reshape(32, 128).T.copy(),
        "w2": np.ascontiguousarray(2 * g2 * np.asarray(W2, f32).T).astype(bf),
        "b2_row": (2 * g2 * np.asarray(b2, f32)).reshape(1, D).astype(bf),
        "ident": np.eye(128, dtype=bf),
    }
    xsplit = {b: _bf16_split(np.ascontiguousarray(x[b].T)) for b in range(B)}

    in_maps = []
    for i in range(NC):
        b, m = i // 4, i % 4
        xh, xl = xsplit[b]
        tiles = [m + 4 * s for s in range(NT)]
        qcols = np.concatenate(
            [np.arange(128 * tt, 128 * (tt + 1)) for tt in tiles])
        mask = np.zeros((NT, 128, 512), f32)
        for s, tt in enumerate(tiles):
            key = 512 * s + np.arange(512)[None, :]
            qabs = 128 * tt + np.arange(128)[:, None]
            mask[s][key > qabs] = NEGBIG
        im = dict(shared)
        im.update({
            "xk_hi": xh, "xk_lo": xl,
            "xq_hi": np.ascontiguousarray(xh[:, qcols]),
            "xq_lo": np.ascontiguousarray(xl[:, qcols]),
            "maskd": mask,
        })
        in_maps.append(im)
    return in_maps


def kernel(**inputs):
    global _compiled
    from concourse.bass_utils import run_bass_kernel_spmd

    if _compiled is None:
        _compiled = _build_program()

    in_maps = _prep_inputs(**inputs)
    res = run_bass_kernel_spmd(_compiled, in_maps, core_ids=list(range(NC)))

    out = np.empty((B, S, D), np.float32)
    for i in range(NC):
        b, m = i // 4, i % 4
        yc = res.results[i]["y"]
        for s in range(NT):
            tt = m + 4 * s
            out[b, 128 * tt:128 * (tt + 1), :] = yc[128 * s:128 * (s + 1), :]
    return out
